# revision 1
# baseline (speedup 1.0000x reference)
"""EpisodicGRU Trainium2 kernel.

Data-parallel over batch: 8 sequences per NeuronCore on 8 cores.
Per core the time recurrence is serial; per step we do
    psum_r = gi_r(t) + W_hr h   (gi precomputed by a big matmul, psum
    psum_z = -(gi_z(t) + W_hz h) initialized via identity-matmul copy)
    psum_n = b_hhn + W_hn h
    r = sigmoid(psum_r); sz = sigmoid(psum_z)          # sz = 1-z
    n = tanh(gi_n(t) + r * psum_n)
    zc = mwneg(t) * sz                                 # -m*w*(1-z)
    h = h*(1+zc) - zc*n
The input-gate GEMM for chunk c+1 is interleaved into the PE idle gaps
of chunk c's recurrence steps.
"""

import os
import sys

for _p in ("/opt/trn_rl_repo", "/root/.axon_site/_ro/trn_rl_repo",
           "/root/.axon_site", "/root/.axon_site/_ro/pypackages"):
    if os.path.isdir(_p) and _p not in sys.path:
        sys.path.append(_p)

import numpy as np
import ml_dtypes

import concourse.bass as bass
import concourse.bacc as bacc
import concourse.tile as tile
from concourse import mybir
from concourse.bass_utils import run_bass_kernel_spmd

F32 = mybir.dt.float32
F32R = mybir.dt.float32r
BF16 = mybir.dt.bfloat16
AF = mybir.ActivationFunctionType
BF16NP = ml_dtypes.bfloat16

B, T_FULL, I, H = 64, 2048, 256, 256
NCORES = 8
BS = B // NCORES          # 8 sequences per core
CH = 128                  # recurrence steps per chunk
GCOLS = BS                # 8 columns per (gate-half)
SCOLS = 2 * GCOLS         # 16 cols per step per gate (2 H-chunks)


def build_nc(T):
    nch = T // CH
    assert T % (2 * CH) == 0
    xflat = (T + 2 * CH) * BS          # padded flat (t, b) length
    mwflat = (T + 2 * CH) * SCOLS

    nc = bacc.Bacc("TRN2", target_bir_lowering=False, debug=False)

    xt_d = nc.dram_tensor("xt", [2, 128, xflat], BF16, kind="ExternalInput").ap()
    mw_d = nc.dram_tensor("mw", [1, mwflat], F32, kind="ExternalInput").ap()
    whT_d = nc.dram_tensor("whT", [12, 128, 128], BF16, kind="ExternalInput").ap()
    wiT_d = nc.dram_tensor("wiT", [12, 128, 128], BF16, kind="ExternalInput").ap()
    brz_d = nc.dram_tensor("brz", [128, 4], F32, kind="ExternalInput").ap()
    bn_d = nc.dram_tensor("bn", [128, 2], F32, kind="ExternalInput").ap()
    bhhn_d = nc.dram_tensor("bhhn", [128, SCOLS], F32R, kind="ExternalInput").ap()
    id_d = nc.dram_tensor("ident", [128, 128], F32R, kind="ExternalInput").ap()
    hout_d = nc.dram_tensor("hout", [128, SCOLS], F32, kind="ExternalOutput").ap()

    with tile.TileContext(nc) as tc:
        consts = tc.alloc_tile_pool(name="consts", bufs=1)
        state = tc.alloc_tile_pool(name="state", bufs=1)
        chunks = tc.alloc_tile_pool(name="chunks", bufs=1)
        temps = tc.alloc_tile_pool(name="temps", bufs=3)
        ps_r_pool = tc.alloc_tile_pool(name="psr", bufs=2, space="PSUM")
        ps_z_pool = tc.alloc_tile_pool(name="psz", bufs=2, space="PSUM")
        ps_n_pool = tc.alloc_tile_pool(name="psn", bufs=2, space="PSUM")
        ps_gi_pool = tc.alloc_tile_pool(name="psgi", bufs=2, space="PSUM")

        # ---- static tiles ----
        whT_s = consts.tile([128, 12 * 128], BF16, tag="whT")
        wiT_s = consts.tile([128, 12 * 128], BF16, tag="wiT")
        brz_s = consts.tile([128, 4], F32, tag="brz")
        bn_s = consts.tile([128, 2], F32, tag="bn")
        bhhn_s = consts.tile([128, SCOLS], F32R, tag="bhhn")
        id_s = consts.tile([128, 128], F32R, tag="ident")
        for t12 in range(12):
            nc.sync.dma_start(whT_s[:, t12 * 128:(t12 + 1) * 128], whT_d[t12])
            nc.sync.dma_start(wiT_s[:, t12 * 128:(t12 + 1) * 128], wiT_d[t12])
        nc.sync.dma_start(brz_s[:], brz_d[:])
        nc.sync.dma_start(bn_s[:], bn_d[:])
        nc.sync.dma_start(bhhn_s[:], bhhn_d[:])
        nc.sync.dma_start(id_s[:], id_d[:])

        h_f = state.tile([128, SCOLS], F32, tag="hf")
        h_b = state.tile([128, SCOLS], BF16, tag="hb")
        nc.vector.memset(h_f[:], 0.0)
        nc.vector.memset(h_b[:], 0.0)

        # ---- per-parity chunk buffers ----
        gi_rz = [chunks.tile([128, CH * 32], F32R, tag=f"girz{p}", name=f"girz{p}")
                 for p in range(2)]
        gin = [chunks.tile([128, CH * SCOLS], F32, tag=f"gin{p}", name=f"gin{p}")
               for p in range(2)]
        mw_s = [chunks.tile([128, CH * SCOLS], F32, tag=f"mw{p}", name=f"mw{p}")
                for p in range(2)]
        xs = [chunks.tile([128, 2 * CH * BS], BF16, tag=f"xs{p}", name=f"xs{p}")
              for p in range(2)]

        def dma_x(par, off_elems):
            # off_elems: flat (t,b) element offset of the chunk
            for half in range(2):
                nc.sync.dma_start(
                    xs[par][:, half * CH * BS:(half + 1) * CH * BS],
                    xt_d[half][:, bass.ds(off_elems, CH * BS)])

        def dma_mw(par, off_elems):
            nc.sync.dma_start(
                mw_s[par][:],
                mw_d[0:1, bass.ds(off_elems, CH * SCOLS)].partition_broadcast(128))

        # Work items producing gi for the chunk living in parity `par`,
        # consuming x from parity `par`.  Returns a list of ("mm"|"cp", thunk)
        # items to interleave into the recurrence steps: one matmul pair or
        # one psum->sbuf copy piece per item.
        def gi_items(par):
            items = []
            for half_n in range(2):        # N-tiles of 512 = 64 steps
                for j in range(6):
                    pg_box = []

                    def mk_mm(jj, nt, box):
                        def emit():
                            pg = ps_gi_pool.tile([128, 512], F32, tag="psgi",
                                                 name="psgi")
                            box.append(pg)
                            for k in range(2):
                                nc.tensor.matmul(
                                    pg[:],
                                    wiT_s[:, (k * 6 + jj) * 128:(k * 6 + jj + 1) * 128],
                                    xs[par][:, k * CH * BS + nt * 512:
                                            k * CH * BS + nt * 512 + 512],
                                    start=(k == 0), stop=(k == 1),
                                    skip_group_check=True)
                        return emit

                    def mk_cp(jj, nt, seg, box):
                        def emit():
                            pg = box[0]
                            pg3 = pg[:].rearrange("p (s b) -> p s b", b=GCOLS)
                            src = pg3[:, seg * 16:(seg + 1) * 16, :]
                            if jj < 4:     # r0,r1,z0,z1 -> gi_rz (bf16)
                                dst = gi_rz[par][:].rearrange(
                                    "p (s g) -> p s g", g=32)[
                                    :, nt * 64 + seg * 16:nt * 64 + (seg + 1) * 16,
                                    jj * GCOLS:(jj + 1) * GCOLS]
                                scale = 1.0 if jj < 2 else -1.0
                                nc.scalar.activation(
                                    dst, src, AF.Identity,
                                    bias=brz_s[:, jj:jj + 1], scale=scale)
                            else:          # n0,n1 -> gin (fp32)
                                jn = jj - 4
                                dst = gin[par][:].rearrange(
                                    "p (s g) -> p s g", g=SCOLS)[
                                    :, nt * 64 + seg * 16:nt * 64 + (seg + 1) * 16,
                                    jn * GCOLS:(jn + 1) * GCOLS]
                                nc.scalar.activation(
                                    dst, src, AF.Identity,
                                    bias=bn_s[:, jn:jn + 1], scale=1.0)
                        return emit

                    items.append(("mm", mk_mm(j, half_n, pg_box)))
                    for seg in range(4):
                        items.append(("cp", mk_cp(j, half_n, seg, pg_box)))
            return items

        W = whT_s

        def emit_step(par, s, mm_item, cp_item):
            ps_r = ps_r_pool.tile([128, SCOLS], F32, tag="psr")
            ps_z = ps_z_pool.tile([128, SCOLS], F32, tag="psz")
            ps_n = ps_n_pool.tile([128, SCOLS], F32, tag="psn")
            # PSUM init via identity matmul (sets has_written for accumulation).
            # float32r operands -> single-pass fp32 matmul (no LOW/HIGH split).
            nc.tensor.matmul(ps_r[:], id_s[:],
                             gi_rz[par][:, s * 32:s * 32 + 16],
                             start=True, stop=False, skip_group_check=True)
            nc.tensor.matmul(ps_z[:], id_s[:],
                             gi_rz[par][:, s * 32 + 16:s * 32 + 32],
                             start=True, stop=False, skip_group_check=True)
            nc.tensor.matmul(ps_n[:], id_s[:], bhhn_s[:],
                             start=True, stop=False, skip_group_check=True)
            # recurrent matmuls; r first (longest downstream chain), z, n
            for (j, dst) in ((0, ps_r), (1, ps_r), (2, ps_z), (3, ps_z),
                             (4, ps_n), (5, ps_n)):
                jj = j % 2
                for k in range(2):
                    nc.tensor.matmul(
                        dst[:, jj * GCOLS:(jj + 1) * GCOLS],
                        W[:, (k * 6 + j) * 128:(k * 6 + j + 1) * 128],
                        h_b[:, k * GCOLS:(k + 1) * GCOLS],
                        start=False, stop=(k == 1 and j in (1, 3, 5)),
                        skip_group_check=True)
            if mm_item:
                mm_item()
            sig_r = temps.tile([128, SCOLS], F32, tag="sigr")
            sig_z = temps.tile([128, SCOLS], F32, tag="sigz")
            t1 = temps.tile([128, SCOLS], F32, tag="t1")
            t2 = temps.tile([128, SCOLS], F32, tag="t2")
            n_t = temps.tile([128, SCOLS], F32, tag="nt")
            zc = temps.tile([128, SCOLS], F32, tag="zc")
            hm = temps.tile([128, SCOLS], F32, tag="hm")
            hzt = temps.tile([128, SCOLS], F32, tag="hzt")
            vv = temps.tile([128, SCOLS], F32, tag="vv")
            ww = temps.tile([128, SCOLS], F32, tag="ww")
            mwt = mw_s[par][:, s * SCOLS:(s + 1) * SCOLS]
            # hm = h*(-m*w): ready at step start, independent of this step's MMs
            nc.vector.tensor_mul(hm[:], h_f[:], mwt)
            nc.scalar.activation(sig_r[:], ps_r[:], AF.Sigmoid)
            nc.scalar.activation(sig_z[:], ps_z[:], AF.Sigmoid)
            # main chain: t1 -> t2 -> tanh -> w -> h
            nc.vector.tensor_mul(t1[:], sig_r[:], ps_n[:])
            nc.vector.tensor_add(t2[:], t1[:], gin[par][:, s * SCOLS:(s + 1) * SCOLS])
            nc.scalar.activation(n_t[:], t2[:], AF.Tanh)
            if cp_item:
                cp_item()
            # z branch (off critical path): v = h*(1-g) = h + (h*(-mw))*sz
            nc.vector.tensor_mul(zc[:], sig_z[:], mwt)
            nc.vector.tensor_mul(hzt[:], hm[:], sig_z[:])
            nc.vector.tensor_add(vv[:], h_f[:], hzt[:])
            # tail: h_new = v - zc*n
            nc.vector.tensor_mul(ww[:], zc[:], n_t[:])
            nc.vector.tensor_sub(h_b[:], vv[:], ww[:])
            nc.vector.tensor_sub(h_f[:], vv[:], ww[:])

        def emit_chunk(par, items):
            mm_q = [th for kind, th in items if kind == "mm"]
            cp_q = [th for kind, th in items if kind == "cp"]
            # order guarantee: copies of mm k must be emitted before mm k+2
            # (psum pool bufs=2).  mm at every 10th step, copies at every
            # other step keeps that order comfortably.
            mi = ci = 0
            for s in range(CH):
                mm_item = None
                cp_item = None
                if s % 10 == 1 and mi < len(mm_q):
                    mm_item = mm_q[mi]
                    mi += 1
                if s % 2 == 0 and ci < len(cp_q) and ci < 4 * mi:
                    cp_item = cp_q[ci]
                    ci += 1
                emit_step(par, s, mm_item, cp_item)
            while mi < len(mm_q):
                mm_q[mi]()
                mi += 1
            while ci < len(cp_q):
                cp_q[ci]()
                ci += 1

        # ---- prologue: x/mw for chunks 0,1 and gi for chunk 0 ----
        dma_x(0, 0)
        dma_x(1, CH * BS)
        dma_mw(0, 0)
        for _kind, th in gi_items(0):
            th()

        # ---- main loop over chunk pairs ----
        if nch > 2:
            assert nch % 2 == 0
            for j in range(nch // 2):
                # xs0 and mw_s1 are free at body start; xs1/mw_s0 are still
                # read during chunk A, so their refills are emitted after it.
                dma_x(0, j * (2 * CH * BS) + 2 * CH * BS)
                dma_mw(1, j * (2 * CH * SCOLS) + CH * SCOLS)
                emit_chunk(0, gi_items(1))
                dma_x(1, j * (2 * CH * BS) + 3 * CH * BS)
                dma_mw(0, j * (2 * CH * SCOLS) + 2 * CH * SCOLS)
                emit_chunk(1, gi_items(0))
        else:
            dma_mw(1, CH * SCOLS)
            emit_chunk(0, gi_items(1))
            emit_chunk(1, [])

        nc.sync.dma_start(hout_d[:], h_f[:])

        for p in (ps_gi_pool, ps_n_pool, ps_z_pool, ps_r_pool, temps,
                  chunks, state, consts):
            p.release()

    nc.compile()
    return nc


def host_prep(x, att_weights, lengths, W_ih, W_hh, b_ih, b_hh, T):
    """Build per-core input maps."""
    xpad = (T + 2 * CH)
    mask = (np.arange(T)[None, :] < np.asarray(lengths)[:, None])
    mwneg = (-(mask * np.asarray(att_weights)[:, :T])).astype(np.float32)  # [B,T]

    Wmod = np.concatenate([W_hh[0:H], -W_hh[H:2 * H], W_hh[2 * H:3 * H]], axis=0)
    whT = np.zeros((12, 128, 128), np.float32)
    wiT = np.zeros((12, 128, 128), np.float32)
    for k in range(2):
        for j in range(6):
            whT[k * 6 + j] = Wmod[j * 128:(j + 1) * 128, k * 128:(k + 1) * 128].T
            wiT[k * 6 + j] = W_ih[j * 128:(j + 1) * 128, k * 128:(k + 1) * 128].T
    whT = whT.astype(BF16NP)
    wiT = wiT.astype(BF16NP)

    bsum = (b_ih + b_hh).astype(np.float32)
    brz = np.zeros((128, 4), np.float32)
    brz[:, 0] = bsum[0:128]
    brz[:, 1] = bsum[128:256]
    brz[:, 2] = -bsum[256:384]
    brz[:, 3] = -bsum[384:512]
    bn = np.zeros((128, 2), np.float32)
    bn[:, 0] = b_ih[512:640]
    bn[:, 1] = b_ih[640:768]
    bhhn = np.zeros((128, SCOLS), np.float32)
    bhhn[:, 0:GCOLS] = np.repeat(b_hh[512:640][:, None], GCOLS, axis=1)
    bhhn[:, GCOLS:SCOLS] = np.repeat(b_hh[640:768][:, None], GCOLS, axis=1)
    ident = np.eye(128, dtype=np.float32)

    in_maps = []
    for c in range(NCORES):
        bs = slice(c * BS, (c + 1) * BS)
        xc = np.asarray(x[bs, :T]).transpose(2, 1, 0)       # [I, T, BS]
        xt = np.zeros((2, 128, xpad * BS), BF16NP)
        xt[:, :, :T * BS] = xc.reshape(2, 128, T * BS).astype(BF16NP)
        mwc = mwneg[bs].T                                    # [T, BS]
        mwt = np.zeros((1, xpad * SCOLS), np.float32)
        mwt[0, :T * SCOLS] = np.concatenate([mwc, mwc], axis=1).reshape(-1)
        in_maps.append({
            "xt": xt, "mw": mwt, "whT": whT, "wiT": wiT,
            "brz": brz, "bn": bn, "bhhn": bhhn, "ident": ident,
        })
    return in_maps


def assemble_out(results):
    out = np.zeros((B, H), np.float32)
    for c, res in enumerate(results):
        ho = res["hout"]                      # [128, 16]
        for k in range(2):
            out[c * BS:(c + 1) * BS, k * 128:(k + 1) * 128] = \
                ho[:, k * GCOLS:(k + 1) * GCOLS].T
    return out


def kernel(x, att_weights, lengths, W_ih, W_hh, b_ih, b_hh):
    x = np.asarray(x)
    in_maps = host_prep(np.asarray(x), np.asarray(att_weights),
                        np.asarray(lengths), np.asarray(W_ih),
                        np.asarray(W_hh), np.asarray(b_ih),
                        np.asarray(b_hh), T_FULL)
    nc = build_nc(T_FULL)
    res = None
    for attempt in range(3):
        try:
            res = run_bass_kernel_spmd(nc, in_maps, core_ids=list(range(NCORES)))
            break
        except Exception:
            if attempt == 2:
                raise
    return assemble_out(res.results)



# revision 3
# speedup vs baseline: 1.1251x; 1.1251x over previous
"""EpisodicGRU Trainium2 kernel.

Data-parallel over batch: 8 sequences per NeuronCore on 8 cores.
Recurrence chain per step (h stored negated, bf16, interleaved odd cols):
    ps_r = gi_r + W_r h            (id MM + 4 rec MMs, bf16)
    sig_r = sigmoid(ps_r)          -> odd cols of sd0 (evens 0)
    t2 = sig_r*ghn + gin           (ONE tensor_tensor_scan over pairs)
    n = tanh(t2)                   -> even cols of ps_n bank
    h' = zc*n + vv                 (ONE tensor_tensor_scan, bf16 out)
where ghn/gin live interleaved in the ps_n psum bank (covering-start
identity MM + strided accumulation), zc = sig(ps_z)*(-m*w) on Pool,
vv = (zc+1)*h on DVE.  Input-gate GEMM and psum->sbuf copies are
interleaved into PE/DVE idle gaps.
"""

import os
import sys

for _p in ("/opt/trn_rl_repo", "/root/.axon_site/_ro/trn_rl_repo",
           "/root/.axon_site", "/root/.axon_site/_ro/pypackages"):
    if os.path.isdir(_p) and _p not in sys.path:
        sys.path.append(_p)

import numpy as np
import ml_dtypes

import concourse.bass as bass
import concourse.bacc as bacc
import concourse.tile as tile
from concourse import mybir
from concourse.bass_utils import run_bass_kernel_spmd

F32 = mybir.dt.float32
BF16 = mybir.dt.bfloat16
AF = mybir.ActivationFunctionType
ALU = mybir.AluOpType
BF16NP = ml_dtypes.bfloat16

B, T_FULL, I, H = 64, 2048, 256, 256
NCORES = 8
BS = B // NCORES          # 8 sequences per core
CH = 128                  # recurrence steps per chunk
GCOLS = BS                # 8 columns per gate-half
SCOLS = 2 * GCOLS         # 16 cols per step (2 H-halves x 8 seqs)


def build_nc(T):
    nch = T // CH
    assert T % CH == 0

    nc = bacc.Bacc("TRN2", target_bir_lowering=False, debug=False)

    xt_d = nc.dram_tensor("xt", [2, 128, T * BS], BF16, kind="ExternalInput").ap()
    mw_d = nc.dram_tensor("mw", [1, T * SCOLS], F32, kind="ExternalInput").ap()
    whT_d = nc.dram_tensor("whT", [12, 128, 128], BF16, kind="ExternalInput").ap()
    wiT_d = nc.dram_tensor("wiT", [12, 128, 128], BF16, kind="ExternalInput").ap()
    brz_d = nc.dram_tensor("brz", [128, 4], F32, kind="ExternalInput").ap()
    bn_d = nc.dram_tensor("bn", [128, 2], F32, kind="ExternalInput").ap()
    bini_d = nc.dram_tensor("bini", [128, CH * 32], BF16, kind="ExternalInput").ap()
    id_d = nc.dram_tensor("ident", [128, 128], BF16, kind="ExternalInput").ap()
    hout_d = nc.dram_tensor("hout", [128, SCOLS], F32, kind="ExternalOutput").ap()

    with tile.TileContext(nc) as tc:
        consts = tc.alloc_tile_pool(name="consts", bufs=1)
        state = tc.alloc_tile_pool(name="state", bufs=1)
        chunks = tc.alloc_tile_pool(name="chunks", bufs=1)
        psr_pool = tc.alloc_tile_pool(name="psr", bufs=2, space="PSUM")
        psz_pool = tc.alloc_tile_pool(name="psz", bufs=2, space="PSUM")
        psn_pool = tc.alloc_tile_pool(name="psn", bufs=2, space="PSUM")
        psgi_pool = tc.alloc_tile_pool(name="psgi", bufs=2, space="PSUM")

        # ---- static tiles ----
        whT_s = consts.tile([128, 12 * 128], BF16, tag="whT")
        wiT_s = consts.tile([128, 12 * 128], BF16, tag="wiT")
        brz_s = consts.tile([128, 4], F32, tag="brz")
        bn_s = consts.tile([128, 2], F32, tag="bn")
        id_s = consts.tile([128, 128], BF16, tag="ident")
        for t12 in range(12):
            nc.sync.dma_start(whT_s[:, t12 * 128:(t12 + 1) * 128], whT_d[t12])
            nc.sync.dma_start(wiT_s[:, t12 * 128:(t12 + 1) * 128], wiT_d[t12])
        nc.sync.dma_start(brz_s[:], brz_d[:])
        nc.sync.dma_start(bn_s[:], bn_d[:])
        nc.sync.dma_start(id_s[:], id_d[:])

        # ---- persistent ping-pong state tiles ----
        ht = [state.tile([128, 32], BF16, tag=f"ht{i}", name=f"ht{i}")
              for i in range(2)]
        sd0 = [state.tile([128, 32], F32, tag=f"sd0{i}", name=f"sd0{i}")
               for i in range(2)]
        zd0 = [state.tile([128, 32], F32, tag=f"zd0{i}", name=f"zd0{i}")
               for i in range(2)]
        szt = [state.tile([128, 16], F32, tag=f"szt{i}", name=f"szt{i}")
               for i in range(2)]
        t2b = [state.tile([128, 32], F32, tag=f"t2b{i}", name=f"t2b{i}")
               for i in range(2)]
        hstage = state.tile([128, SCOLS], F32, tag="hstage")
        for i in range(2):
            nc.vector.memset(ht[i][:], 0.0)
            nc.vector.memset(sd0[i][:], 0.0)
            nc.vector.memset(zd0[i][:], 0.0)

        # ---- per-parity chunk buffers ----
        girz = [chunks.tile([128, CH * 32], BF16, tag=f"girz{p}", name=f"girz{p}")
                for p in range(2)]
        gn = [chunks.tile([128, CH * 32], BF16, tag=f"gn{p}", name=f"gn{p}")
              for p in range(2)]
        mw_s = [chunks.tile([128, CH * SCOLS], F32, tag=f"mw{p}", name=f"mw{p}")
                for p in range(2)]
        xs = [chunks.tile([128, 2 * CH * BS], BF16, tag=f"xs{p}", name=f"xs{p}")
              for p in range(2)]
        # gn evens = bhhn pattern (constant), odds overwritten per chunk
        for p in range(2):
            nc.sync.dma_start(gn[p][:], bini_d[:])

        def dma_x(par, c):
            off = c * CH * BS
            for half in range(2):
                nc.sync.dma_start(
                    xs[par][:, half * CH * BS:(half + 1) * CH * BS],
                    xt_d[half][:, bass.ds(off, CH * BS)])

        def dma_mw(par, c):
            nc.sync.dma_start(
                mw_s[par][:],
                mw_d[0:1, bass.ds(c * CH * SCOLS, CH * SCOLS)]
                .partition_broadcast(128))

        # gi work items for the chunk living in parity `par`:
        # 24 "mm" items (one matmul each) + 48 "cp" items (one DVE
        # tensor_scalar each).
        def gi_items(par):
            mm_q = []
            cp_q = []
            for half_n in range(2):            # 64-step halves of the chunk
                for j in range(6):
                    pg_box = []

                    def mk_mm(jj, nt, kk, box):
                        def emit():
                            if kk == 0:
                                pg = psgi_pool.tile([128, 512], F32, tag="psgi",
                                                    name="psgi")
                                box.append(pg)
                            pg = box[0]
                            nc.tensor.matmul(
                                pg[:],
                                wiT_s[:, (kk * 6 + jj) * 128:(kk * 6 + jj + 1) * 128],
                                xs[par][:, kk * CH * BS + nt * 512:
                                        kk * CH * BS + nt * 512 + 512],
                                start=(kk == 0), stop=(kk == 1),
                                skip_group_check=True)
                        return emit

                    def mk_cp(jj, nt, seg, box):
                        def emit():
                            pg = box[0]
                            pg3 = pg[:].rearrange("p (s b) -> p s b", b=GCOLS)
                            src = pg3[:, seg * 16:(seg + 1) * 16, :]
                            s0 = nt * 64 + seg * 16
                            if jj < 4:     # r0,r1,z0,z1 -> girz
                                dst = girz[par][:].rearrange(
                                    "p (s g) -> p s g", g=32)[
                                    :, s0:s0 + 16, jj * GCOLS:(jj + 1) * GCOLS]
                                scale = 1.0 if jj < 2 else -1.0
                                nc.vector.tensor_scalar(
                                    dst, src, brz_s[:, jj:jj + 1], scale,
                                    op0=ALU.add, op1=ALU.mult)
                            else:          # n0,n1 -> gn odd cols
                                jn = jj - 4
                                dst = gn[par][:].rearrange(
                                    "p (s g x) -> p s g x", g=16, x=2)[
                                    :, s0:s0 + 16, jn * GCOLS:(jn + 1) * GCOLS,
                                    1]
                                nc.vector.tensor_scalar(
                                    dst, src, bn_s[:, jn:jn + 1], 1.0,
                                    op0=ALU.add, op1=ALU.mult)
                        return emit

                    for kk in range(2):
                        mm_q.append(mk_mm(j, half_n, kk, pg_box))
                    for seg in range(4):
                        cp_q.append(mk_cp(j, half_n, seg, pg_box))
            return mm_q, cp_q

        W = whT_s

        def emit_step(par, cs, s, mm_item, cp_items):
            pr = s % 2
            ps_r = psr_pool.tile([128, SCOLS], F32, tag="psr")
            ps_z = psz_pool.tile([128, SCOLS], F32, tag="psz")
            ps_n = psn_pool.tile([128, 32], F32, tag="psn")
            hv = ht[pr][:].rearrange("p (i x) -> p i x", x=2)
            # identity matmuls: psum init from sbuf (bf16, cheap)
            nc.tensor.matmul(ps_r[:], id_s[:],
                             girz[par][:, cs * 32:cs * 32 + 16],
                             start=True, stop=False, skip_group_check=True)
            nc.tensor.matmul(ps_z[:], id_s[:],
                             girz[par][:, cs * 32 + 16:cs * 32 + 32],
                             start=True, stop=False, skip_group_check=True)
            nc.tensor.matmul(ps_n[:], id_s[:],
                             gn[par][:, cs * 32:(cs + 1) * 32],
                             start=True, stop=False, skip_group_check=True)
            # recurrent matmuls: r first (critical path), then z, then n
            psnv = ps_n[:].rearrange("p (i x) -> p i x", x=2)
            for (j, jj, dst) in ((0, 0, ps_r), (1, 1, ps_r),
                                 (2, 0, ps_z), (3, 1, ps_z)):
                for k in range(2):
                    nc.tensor.matmul(
                        dst[:, jj * GCOLS:(jj + 1) * GCOLS],
                        W[:, (k * 6 + j) * 128:(k * 6 + j + 1) * 128],
                        hv[:, k * GCOLS:(k + 1) * GCOLS, 1],
                        start=False, stop=(k == 1 and j in (1, 3)),
                        skip_group_check=True)
            for j in (4, 5):
                jj = j - 4
                for k in range(2):
                    nc.tensor.matmul(
                        psnv[:, jj * GCOLS:(jj + 1) * GCOLS, 0:1],
                        W[:, (k * 6 + j) * 128:(k * 6 + j + 1) * 128],
                        hv[:, k * GCOLS:(k + 1) * GCOLS, 1:2],
                        start=False, stop=(k == 1 and j == 5),
                        skip_group_check=True)
            if mm_item:
                mm_item()
            # Act: sig_r -> sd0 odds; sig_z -> szt; (tanh later)
            sd0v = sd0[pr][:].rearrange("p (i x) -> p i x", x=2)
            nc.scalar.activation(sd0v[:, :, 1:2],
                                 ps_r[:].rearrange("p (i x) -> p i x", x=1),
                                 AF.Sigmoid)
            nc.scalar.activation(szt[pr][:], ps_z[:], AF.Sigmoid)
            # DVE: scan1: t2 = sig_r*ghn + gin (odds of t2b)
            nc.vector.tensor_tensor_scan(
                t2b[pr][:], sd0[pr][:], ps_n[:], 0.0,
                op0=ALU.mult, op1=ALU.add)
            # Act: tanh(t2) -> ps_n evens
            t2v = t2b[pr][:].rearrange("p (i x) -> p i x", x=2)
            nc.scalar.activation(psnv[:, :, 0:1], t2v[:, :, 1:2], AF.Tanh)
            # Pool: zc = sig_z * (-m*w) -> zd0 odds
            zd0v = zd0[pr][:].rearrange("p (i x) -> p i x", x=2)
            mwv = mw_s[par][:, cs * SCOLS:(cs + 1) * SCOLS] \
                .rearrange("p (i x) -> p i x", x=1)
            nc.gpsimd.tensor_tensor(
                zd0v[:, :, 1:2],
                szt[pr][:].rearrange("p (i x) -> p i x", x=1),
                mwv, op=ALU.mult)
            # DVE: vv = (zc+1)*h -> ps_n odds
            nc.vector.scalar_tensor_tensor(
                psnv[:, :, 1:2], zd0v[:, :, 1:2], 1.0, hv[:, :, 1:2],
                op0=ALU.add, op1=ALU.mult)
            # DVE: scan2: h' = zc*n + vv  (bf16, odds of ht[1-pr])
            nc.vector.tensor_tensor_scan(
                ht[1 - pr][:], zd0[pr][:], ps_n[:], 0.0,
                op0=ALU.mult, op1=ALU.add)
            for cp in cp_items:
                cp()

        def emit_chunk(par, s0, gi):
            mm_q, cp_q = gi
            mi = ci = 0
            for cs in range(CH):
                mm_item = None
                if cs % 5 == 1 and mi < len(mm_q):
                    mm_item = mm_q[mi]
                    mi += 1
                cps = []
                if cs % 2 == 0:
                    lim = 4 * (mi // 2)
                    while ci < len(cp_q) and ci < lim and len(cps) < 2:
                        cps.append(cp_q[ci])
                        ci += 1
                emit_step(par, cs, s0 + cs, mm_item, cps)
            while mi < len(mm_q):
                mm_q[mi]()
                mi += 1
            while ci < len(cp_q):
                cp_q[ci]()
                ci += 1

        # ---- prologue ----
        dma_x(0, 0)
        dma_mw(0, 0)
        if nch > 1:
            dma_x(1, 1)
        mm0, cp0 = gi_items(0)
        for i, mm in enumerate(mm0):
            mm()
            if i % 2 == 1:
                for cp in cp0[(i // 2) * 4:(i // 2) * 4 + 4]:
                    cp()

        # ---- main loop over chunks ----
        for c in range(nch):
            par = c % 2
            if c + 2 < nch:
                dma_x(par, c + 2)
            if c + 1 < nch:
                dma_mw(1 - par, c + 1)
            gi = gi_items(1 - par) if c + 1 < nch else ([], [])
            emit_chunk(par, c * CH, gi)

        # ---- output: h_true = -h_store ----
        hfv = ht[T % 2][:].rearrange("p (i x) -> p i x", x=2)
        nc.scalar.activation(hstage[:].rearrange("p (i x) -> p i x", x=1),
                             hfv[:, :, 1:2], AF.Copy, scale=-1.0)
        nc.sync.dma_start(hout_d[:], hstage[:])

        for p in (psgi_pool, psn_pool, psz_pool, psr_pool,
                  chunks, state, consts):
            p.release()

    nc.compile()
    return nc


def host_prep(x, att_weights, lengths, W_ih, W_hh, b_ih, b_hh, T):
    """Build per-core input maps."""
    mask = (np.arange(T)[None, :] < np.asarray(lengths)[:, None])
    mwneg = (-(mask * np.asarray(att_weights)[:, :T])).astype(np.float32)

    # h is stored negated: negate r and n weight blocks (z stays positive
    # because ps_z accumulates -a_z).
    Wmod = np.concatenate([-W_hh[0:H], W_hh[H:2 * H], -W_hh[2 * H:3 * H]],
                          axis=0)
    whT = np.zeros((12, 128, 128), np.float32)
    wiT = np.zeros((12, 128, 128), np.float32)
    for k in range(2):
        for j in range(6):
            whT[k * 6 + j] = Wmod[j * 128:(j + 1) * 128, k * 128:(k + 1) * 128].T
            wiT[k * 6 + j] = W_ih[j * 128:(j + 1) * 128, k * 128:(k + 1) * 128].T
    whT = whT.astype(BF16NP)
    wiT = wiT.astype(BF16NP)

    bsum = (b_ih + b_hh).astype(np.float32)
    brz = np.zeros((128, 4), np.float32)
    brz[:, 0] = bsum[0:128]
    brz[:, 1] = bsum[128:256]
    brz[:, 2] = bsum[256:384]
    brz[:, 3] = bsum[384:512]
    bn = np.zeros((128, 2), np.float32)
    bn[:, 0] = b_ih[512:640]
    bn[:, 1] = b_ih[640:768]
    # gn init pattern: evens = bhh_n (per half), odds = 0
    pat = np.zeros((128, 32), np.float32)
    for jj in range(2):
        pat[:, 2 * (jj * GCOLS) + np.arange(GCOLS) * 2] = \
            b_hh[512 + jj * 128:512 + (jj + 1) * 128][:, None]
    bini = np.tile(pat, (1, CH)).astype(BF16NP)
    ident = np.eye(128, dtype=np.float32).astype(BF16NP)

    in_maps = []
    for c in range(NCORES):
        bs = slice(c * BS, (c + 1) * BS)
        xc = np.asarray(x[bs, :T]).transpose(2, 1, 0)       # [I, T, BS]
        xt = xc.reshape(2, 128, T * BS).astype(BF16NP)
        mwc = mwneg[bs].T                                    # [T, BS]
        mwt = np.concatenate([mwc, mwc], axis=1).reshape(1, T * SCOLS)
        in_maps.append({
            "xt": xt, "mw": np.ascontiguousarray(mwt), "whT": whT,
            "wiT": wiT, "brz": brz, "bn": bn, "bini": bini, "ident": ident,
        })
    return in_maps


def assemble_out(results):
    out = np.zeros((B, H), np.float32)
    for c, res in enumerate(results):
        ho = res["hout"]                      # [128, 16]
        for k in range(2):
            out[c * BS:(c + 1) * BS, k * 128:(k + 1) * 128] = \
                ho[:, k * GCOLS:(k + 1) * GCOLS].T
    return out


def run_T(lengths):
    mx = int(np.asarray(lengths).max())
    nch = max(1, -(-mx // CH))
    return min(nch * CH, T_FULL)


def kernel(x, att_weights, lengths, W_ih, W_hh, b_ih, b_hh):
    T = run_T(lengths)
    in_maps = host_prep(np.asarray(x), np.asarray(att_weights),
                        np.asarray(lengths), np.asarray(W_ih),
                        np.asarray(W_hh), np.asarray(b_ih),
                        np.asarray(b_hh), T)
    nc = build_nc(T)
    res = None
    for attempt in range(3):
        try:
            res = run_bass_kernel_spmd(nc, in_maps, core_ids=list(range(NCORES)))
            break
        except Exception:
            if attempt == 2:
                raise
    return assemble_out(res.results)


# revision 5
# speedup vs baseline: 1.2631x; 1.1226x over previous
"""EpisodicGRU Trainium2 kernel.

Data-parallel over batch: 8 sequences per NeuronCore on 8 cores.
Recurrence chain per step (h stored negated, bf16, interleaved odd cols):
    ps_r = gi_r + W_r h            (id MM + 4 rec MMs, bf16)
    sig_r = sigmoid(ps_r)          -> odd cols of sd0 (evens 0)
    t2 = sig_r*ghn + gin           (ONE tensor_tensor_scan over pairs)
    n = tanh(t2)                   -> even cols of ps_n bank
    h' = zc*n + vv                 (ONE tensor_tensor_scan, bf16 out)
where ghn/gin live interleaved in the ps_n psum bank (covering-start
identity MM + strided accumulation), zc = sig(ps_z)*(-m*w) on Pool,
vv = (zc+1)*h on DVE.  Input-gate GEMM and psum->sbuf copies are
interleaved into PE/DVE idle gaps.
"""

import os
import sys

for _p in ("/opt/trn_rl_repo", "/root/.axon_site/_ro/trn_rl_repo",
           "/root/.axon_site", "/root/.axon_site/_ro/pypackages"):
    if os.path.isdir(_p) and _p not in sys.path:
        sys.path.append(_p)

import numpy as np
import ml_dtypes

import concourse.bass as bass
import concourse.bacc as bacc
import concourse.tile as tile
from concourse import mybir
from concourse.bass_utils import run_bass_kernel_spmd

F32 = mybir.dt.float32
BF16 = mybir.dt.bfloat16
AF = mybir.ActivationFunctionType
ALU = mybir.AluOpType
BF16NP = ml_dtypes.bfloat16

B, T_FULL, I, H = 64, 2048, 256, 256
NCORES = 8
BS = B // NCORES          # 8 sequences per core
CH = 128                  # recurrence steps per chunk
GCOLS = BS                # 8 columns per gate-half
SCOLS = 2 * GCOLS         # 16 cols per step (2 H-halves x 8 seqs)


def build_nc(T):
    nch = T // CH
    assert T % CH == 0

    nc = bacc.Bacc("TRN2", target_bir_lowering=False, debug=False)

    xt_d = nc.dram_tensor("xt", [2, 128, T * BS], BF16, kind="ExternalInput").ap()
    mw_d = nc.dram_tensor("mw", [1, T * SCOLS], F32, kind="ExternalInput").ap()
    whT_d = nc.dram_tensor("whT", [12, 128, 128], BF16, kind="ExternalInput").ap()
    wiT_d = nc.dram_tensor("wiT", [12, 128, 128], BF16, kind="ExternalInput").ap()
    brz_d = nc.dram_tensor("brz", [128, 4], F32, kind="ExternalInput").ap()
    bn_d = nc.dram_tensor("bn", [128, 2], F32, kind="ExternalInput").ap()
    bini_d = nc.dram_tensor("bini", [128, CH * 32], BF16, kind="ExternalInput").ap()
    id_d = nc.dram_tensor("ident", [128, 128], BF16, kind="ExternalInput").ap()
    hout_d = nc.dram_tensor("hout", [128, SCOLS], F32, kind="ExternalOutput").ap()

    with tile.TileContext(nc) as tc:
        consts = tc.alloc_tile_pool(name="consts", bufs=1)
        state = tc.alloc_tile_pool(name="state", bufs=1)
        chunks = tc.alloc_tile_pool(name="chunks", bufs=1)
        psr_pool = tc.alloc_tile_pool(name="psr", bufs=2, space="PSUM")
        psz_pool = tc.alloc_tile_pool(name="psz", bufs=2, space="PSUM")
        psn_pool = tc.alloc_tile_pool(name="psn", bufs=2, space="PSUM")
        psgi_pool = tc.alloc_tile_pool(name="psgi", bufs=2, space="PSUM")

        # ---- static tiles ----
        whT_s = consts.tile([128, 12 * 128], BF16, tag="whT")
        wiT_s = consts.tile([128, 12 * 128], BF16, tag="wiT")
        brz_s = consts.tile([128, 4], F32, tag="brz")
        bn_s = consts.tile([128, 2], F32, tag="bn")
        id_s = consts.tile([128, 128], BF16, tag="ident")
        for t12 in range(12):
            nc.sync.dma_start(whT_s[:, t12 * 128:(t12 + 1) * 128], whT_d[t12])
            nc.sync.dma_start(wiT_s[:, t12 * 128:(t12 + 1) * 128], wiT_d[t12])
        nc.sync.dma_start(brz_s[:], brz_d[:])
        nc.sync.dma_start(bn_s[:], bn_d[:])
        nc.sync.dma_start(id_s[:], id_d[:])

        # ---- persistent ping-pong state tiles ----
        ht = [state.tile([128, 32], BF16, tag=f"ht{i}", name=f"ht{i}")
              for i in range(2)]
        sd0 = [state.tile([128, 32], F32, tag=f"sd0{i}", name=f"sd0{i}")
               for i in range(2)]
        nt = [state.tile([128, 32], F32, tag=f"nt{i}", name=f"nt{i}")
              for i in range(2)]
        zv = [state.tile([128, 32], F32, tag=f"zv{i}", name=f"zv{i}")
              for i in range(2)]
        szt = [state.tile([128, 16], F32, tag=f"szt{i}", name=f"szt{i}")
               for i in range(2)]
        t2b = [state.tile([128, 32], F32, tag=f"t2b{i}", name=f"t2b{i}")
               for i in range(2)]
        hstage = state.tile([128, SCOLS], F32, tag="hstage")
        for i in range(2):
            nc.vector.memset(ht[i][:], 0.0)
            nc.vector.memset(sd0[i][:], 0.0)
            nc.vector.memset(nt[i][:], 0.0)

        # ---- per-parity chunk buffers ----
        girz = [chunks.tile([128, CH * 32], BF16, tag=f"girz{p}", name=f"girz{p}")
                for p in range(2)]
        gn = [chunks.tile([128, CH * 32], BF16, tag=f"gn{p}", name=f"gn{p}")
              for p in range(2)]
        mw_s = [chunks.tile([128, CH * SCOLS], F32, tag=f"mw{p}", name=f"mw{p}")
                for p in range(2)]
        xs = [chunks.tile([128, 2 * CH * BS], BF16, tag=f"xs{p}", name=f"xs{p}")
              for p in range(2)]
        # gn evens = bhhn pattern (constant), odds overwritten per chunk
        for p in range(2):
            nc.sync.dma_start(gn[p][:], bini_d[:])

        def dma_x(par, c):
            off = c * CH * BS
            for half in range(2):
                nc.sync.dma_start(
                    xs[par][:, half * CH * BS:(half + 1) * CH * BS],
                    xt_d[half][:, bass.ds(off, CH * BS)])

        def dma_mw(par, c):
            nc.sync.dma_start(
                mw_s[par][:],
                mw_d[0:1, bass.ds(c * CH * SCOLS, CH * SCOLS)]
                .partition_broadcast(128))

        # gi work items for the chunk living in parity `par`:
        # 24 "mm" items (one matmul each) + 48 "cp" items (one DVE
        # tensor_scalar each).
        def gi_items(par):
            mm_q = []
            cp_q = []
            for half_n in range(2):            # 64-step halves of the chunk
                for j in range(6):
                    pg_box = []

                    def mk_mm(jj, nt, kk, box):
                        def emit():
                            if kk == 0:
                                pg = psgi_pool.tile([128, 512], F32, tag="psgi",
                                                    name="psgi")
                                box.append(pg)
                            pg = box[0]
                            nc.tensor.matmul(
                                pg[:],
                                wiT_s[:, (kk * 6 + jj) * 128:(kk * 6 + jj + 1) * 128],
                                xs[par][:, kk * CH * BS + nt * 512:
                                        kk * CH * BS + nt * 512 + 512],
                                start=(kk == 0), stop=(kk == 1),
                                skip_group_check=True)
                        return emit

                    def mk_cp(jj, nt, seg, box):
                        def emit():
                            pg = box[0]
                            pg3 = pg[:].rearrange("p (s b) -> p s b", b=GCOLS)
                            src = pg3[:, seg * 16:(seg + 1) * 16, :]
                            s0 = nt * 64 + seg * 16
                            if jj < 4:     # r0,r1,z0,z1 -> girz
                                dst = girz[par][:].rearrange(
                                    "p (s g) -> p s g", g=32)[
                                    :, s0:s0 + 16, jj * GCOLS:(jj + 1) * GCOLS]
                                scale = 1.0 if jj < 2 else -1.0
                                nc.vector.tensor_scalar(
                                    dst, src, brz_s[:, jj:jj + 1], scale,
                                    op0=ALU.add, op1=ALU.mult)
                            else:          # n0,n1 -> gn odd cols
                                jn = jj - 4
                                dst = gn[par][:].rearrange(
                                    "p (s g x) -> p s g x", g=16, x=2)[
                                    :, s0:s0 + 16, jn * GCOLS:(jn + 1) * GCOLS,
                                    1]
                                nc.vector.tensor_scalar(
                                    dst, src, bn_s[:, jn:jn + 1], 1.0,
                                    op0=ALU.add, op1=ALU.mult)
                        return emit

                    for kk in range(2):
                        mm_q.append(mk_mm(j, half_n, kk, pg_box))
                    for seg in range(4):
                        cp_q.append(mk_cp(j, half_n, seg, pg_box))
            return mm_q, cp_q

        W = whT_s

        def emit_step(par, cs, s, mm_item, cp_items):
            pr = s % 2
            ps_r = psr_pool.tile([128, SCOLS], F32, tag="psr")
            ps_z = psz_pool.tile([128, SCOLS], F32, tag="psz")
            ps_n = psn_pool.tile([128, 32], F32, tag="psn")
            hv = ht[pr][:].rearrange("p (i x) -> p i x", x=2)
            # identity matmuls: psum init from sbuf (bf16, cheap)
            nc.tensor.matmul(ps_r[:], id_s[:],
                             girz[par][:, cs * 32:cs * 32 + 16],
                             start=True, stop=False, skip_group_check=True)
            nc.tensor.matmul(ps_z[:], id_s[:],
                             girz[par][:, cs * 32 + 16:cs * 32 + 32],
                             start=True, stop=False, skip_group_check=True)
            nc.tensor.matmul(ps_n[:], id_s[:],
                             gn[par][:, cs * 32:(cs + 1) * 32],
                             start=True, stop=False, skip_group_check=True)
            # recurrent matmuls: r first (critical path), then z, then n
            psnv = ps_n[:].rearrange("p (i x) -> p i x", x=2)
            for (j, jj, dst) in ((0, 0, ps_r), (1, 1, ps_r),
                                 (2, 0, ps_z), (3, 1, ps_z)):
                for k in range(2):
                    nc.tensor.matmul(
                        dst[:, jj * GCOLS:(jj + 1) * GCOLS],
                        W[:, (k * 6 + j) * 128:(k * 6 + j + 1) * 128],
                        hv[:, k * GCOLS:(k + 1) * GCOLS, 1],
                        start=False, stop=(k == 1 and j in (1, 3)),
                        skip_group_check=True)
            for j in (4, 5):
                jj = j - 4
                for k in range(2):
                    nc.tensor.matmul(
                        psnv[:, jj * GCOLS:(jj + 1) * GCOLS, 0:1],
                        W[:, (k * 6 + j) * 128:(k * 6 + j + 1) * 128],
                        hv[:, k * GCOLS:(k + 1) * GCOLS, 1:2],
                        start=False, stop=(k == 1 and j == 5),
                        skip_group_check=True)
            if mm_item:
                mm_item()
            # Act: sig_r -> sd0 odds; sig_z -> szt; (tanh later)
            sd0v = sd0[pr][:].rearrange("p (i x) -> p i x", x=2)
            nc.scalar.activation(sd0v[:, :, 1:2],
                                 ps_r[:].rearrange("p (i x) -> p i x", x=1),
                                 AF.Sigmoid)
            nc.scalar.activation(szt[pr][:], ps_z[:], AF.Sigmoid)
            # DVE: scan1: t2 = sig_r*ghn + gin (odds of t2b)
            nc.vector.tensor_tensor_scan(
                t2b[pr][:], sd0[pr][:], ps_n[:], 0.0,
                op0=ALU.mult, op1=ALU.add)
            # Act: tanh(t2) -> nt odds (sole writer of nt)
            t2v = t2b[pr][:].rearrange("p (i x) -> p i x", x=2)
            ntv = nt[pr][:].rearrange("p (i x) -> p i x", x=2)
            nc.scalar.activation(ntv[:, :, 1:2], t2v[:, :, 1:2], AF.Tanh)
            # DVE: zc = sig_z * (-m*w) -> zv evens
            zvv = zv[pr][:].rearrange("p (i x) -> p i x", x=2)
            mwv = mw_s[par][:, cs * SCOLS:(cs + 1) * SCOLS] \
                .rearrange("p (i x) -> p i x", x=1)
            nc.vector.tensor_tensor(
                zvv[:, :, 0:1],
                szt[pr][:].rearrange("p (i x) -> p i x", x=1),
                mwv, op=ALU.mult)
            # DVE: vv = (zc+1)*h -> zv odds
            nc.vector.scalar_tensor_tensor(
                zvv[:, :, 1:2], zvv[:, :, 0:1], 1.0, hv[:, :, 1:2],
                op0=ALU.add, op1=ALU.mult)
            # DVE: scan2: pairs (0,n)x(zc,vv): h' = n*zc + vv (bf16 odds)
            nc.vector.tensor_tensor_scan(
                ht[1 - pr][:], nt[pr][:], zv[pr][:], 0.0,
                op0=ALU.mult, op1=ALU.add)
            for cp in cp_items:
                cp()

        def emit_chunk(par, s0, gi):
            mm_q, cp_q = gi
            mi = ci = 0
            for cs in range(CH):
                mm_item = None
                if cs % 5 == 1 and mi < len(mm_q):
                    mm_item = mm_q[mi]
                    mi += 1
                cps = []
                if cs % 2 == 0:
                    lim = 4 * (mi // 2)
                    while ci < len(cp_q) and ci < lim and len(cps) < 2:
                        cps.append(cp_q[ci])
                        ci += 1
                emit_step(par, cs, s0 + cs, mm_item, cps)
            while mi < len(mm_q):
                mm_q[mi]()
                mi += 1
            while ci < len(cp_q):
                cp_q[ci]()
                ci += 1

        # ---- prologue ----
        dma_x(0, 0)
        dma_mw(0, 0)
        if nch > 1:
            dma_x(1, 1)
        mm0, cp0 = gi_items(0)
        for i, mm in enumerate(mm0):
            mm()
            if i % 2 == 1:
                for cp in cp0[(i // 2) * 4:(i // 2) * 4 + 4]:
                    cp()

        # ---- main loop over chunks ----
        for c in range(nch):
            par = c % 2
            if c + 2 < nch:
                dma_x(par, c + 2)
            if c + 1 < nch:
                dma_mw(1 - par, c + 1)
            gi = gi_items(1 - par) if c + 1 < nch else ([], [])
            emit_chunk(par, c * CH, gi)

        # ---- output: h_true = -h_store ----
        hfv = ht[T % 2][:].rearrange("p (i x) -> p i x", x=2)
        nc.scalar.activation(hstage[:].rearrange("p (i x) -> p i x", x=1),
                             hfv[:, :, 1:2], AF.Copy, scale=-1.0)
        nc.sync.dma_start(hout_d[:], hstage[:])

        for p in (psgi_pool, psn_pool, psz_pool, psr_pool,
                  chunks, state, consts):
            p.release()

    nc.compile()
    return nc


def host_prep(x, att_weights, lengths, W_ih, W_hh, b_ih, b_hh, T):
    """Build per-core input maps."""
    mask = (np.arange(T)[None, :] < np.asarray(lengths)[:, None])
    mwneg = (-(mask * np.asarray(att_weights)[:, :T])).astype(np.float32)

    # h is stored negated: negate r and n weight blocks (z stays positive
    # because ps_z accumulates -a_z).
    Wmod = np.concatenate([-W_hh[0:H], W_hh[H:2 * H], -W_hh[2 * H:3 * H]],
                          axis=0)
    whT = np.zeros((12, 128, 128), np.float32)
    wiT = np.zeros((12, 128, 128), np.float32)
    for k in range(2):
        for j in range(6):
            whT[k * 6 + j] = Wmod[j * 128:(j + 1) * 128, k * 128:(k + 1) * 128].T
            wiT[k * 6 + j] = W_ih[j * 128:(j + 1) * 128, k * 128:(k + 1) * 128].T
    whT = whT.astype(BF16NP)
    wiT = wiT.astype(BF16NP)

    bsum = (b_ih + b_hh).astype(np.float32)
    brz = np.zeros((128, 4), np.float32)
    brz[:, 0] = bsum[0:128]
    brz[:, 1] = bsum[128:256]
    brz[:, 2] = bsum[256:384]
    brz[:, 3] = bsum[384:512]
    bn = np.zeros((128, 2), np.float32)
    bn[:, 0] = b_ih[512:640]
    bn[:, 1] = b_ih[640:768]
    # gn init pattern: evens = bhh_n (per half), odds = 0
    pat = np.zeros((128, 32), np.float32)
    for jj in range(2):
        pat[:, 2 * (jj * GCOLS) + np.arange(GCOLS) * 2] = \
            b_hh[512 + jj * 128:512 + (jj + 1) * 128][:, None]
    bini = np.tile(pat, (1, CH)).astype(BF16NP)
    ident = np.eye(128, dtype=np.float32).astype(BF16NP)

    in_maps = []
    for c in range(NCORES):
        bs = slice(c * BS, (c + 1) * BS)
        xc = np.asarray(x[bs, :T]).transpose(2, 1, 0)       # [I, T, BS]
        xt = xc.reshape(2, 128, T * BS).astype(BF16NP)
        mwc = mwneg[bs].T                                    # [T, BS]
        mwt = np.concatenate([mwc, mwc], axis=1).reshape(1, T * SCOLS)
        in_maps.append({
            "xt": xt, "mw": np.ascontiguousarray(mwt), "whT": whT,
            "wiT": wiT, "brz": brz, "bn": bn, "bini": bini, "ident": ident,
        })
    return in_maps


def assemble_out(results):
    out = np.zeros((B, H), np.float32)
    for c, res in enumerate(results):
        ho = res["hout"]                      # [128, 16]
        for k in range(2):
            out[c * BS:(c + 1) * BS, k * 128:(k + 1) * 128] = \
                ho[:, k * GCOLS:(k + 1) * GCOLS].T
    return out


def run_T(lengths):
    mx = int(np.asarray(lengths).max())
    nch = max(1, -(-mx // CH))
    return min(nch * CH, T_FULL)


def kernel(x, att_weights, lengths, W_ih, W_hh, b_ih, b_hh):
    T = run_T(lengths)
    in_maps = host_prep(np.asarray(x), np.asarray(att_weights),
                        np.asarray(lengths), np.asarray(W_ih),
                        np.asarray(W_hh), np.asarray(b_ih),
                        np.asarray(b_hh), T)
    nc = build_nc(T)
    res = None
    for attempt in range(3):
        try:
            res = run_bass_kernel_spmd(nc, in_maps, core_ids=list(range(NCORES)))
            break
        except Exception:
            if attempt == 2:
                raise
    return assemble_out(res.results)


# revision 9
# speedup vs baseline: 1.2671x; 1.0032x over previous
"""EpisodicGRU Trainium2 kernel.

Data-parallel over batch: 8 sequences per NeuronCore on 8 cores.
Recurrence chain per step (h stored negated, bf16, interleaved odd cols):
    ps_r = gi_r + W_r h            (id MM + 4 rec MMs, bf16)
    sig_r = sigmoid(ps_r)          -> odd cols of sd0 (evens 0)
    t2 = sig_r*ghn + gin           (ONE tensor_tensor_scan over pairs)
    n = tanh(t2)                   -> even cols of ps_n bank
    h' = zc*n + vv                 (ONE tensor_tensor_scan, bf16 out)
where ghn/gin live interleaved in the ps_n psum bank (covering-start
identity MM + strided accumulation), zc = sig(ps_z)*(-m*w) on Pool,
vv = (zc+1)*h on DVE.  Input-gate GEMM and psum->sbuf copies are
interleaved into PE/DVE idle gaps.
"""

import os
import sys

for _p in ("/opt/trn_rl_repo", "/root/.axon_site/_ro/trn_rl_repo",
           "/root/.axon_site", "/root/.axon_site/_ro/pypackages"):
    if os.path.isdir(_p) and _p not in sys.path:
        sys.path.append(_p)

import numpy as np
import ml_dtypes

import concourse.bass as bass
import concourse.bacc as bacc
import concourse.tile as tile
from concourse import mybir
from concourse.bass_utils import run_bass_kernel_spmd

F32 = mybir.dt.float32
BF16 = mybir.dt.bfloat16
AF = mybir.ActivationFunctionType
ALU = mybir.AluOpType
BF16NP = ml_dtypes.bfloat16

B, T_FULL, I, H = 64, 2048, 256, 256
NCORES = 8
BS = B // NCORES          # 8 sequences per core
CH = 128                  # recurrence steps per chunk
GCOLS = BS                # 8 columns per gate-half
SCOLS = 2 * GCOLS         # 16 cols per step (2 H-halves x 8 seqs)


def build_nc(T):
    nch = T // CH
    assert T % CH == 0

    nc = bacc.Bacc("TRN2", target_bir_lowering=False, debug=False)

    xt_d = nc.dram_tensor("xt", [2, 128, T * BS], BF16, kind="ExternalInput").ap()
    mw_d = nc.dram_tensor("mw", [1, T * SCOLS], F32, kind="ExternalInput").ap()
    whT_d = nc.dram_tensor("whT", [12, 128, 128], BF16, kind="ExternalInput").ap()
    wiT_d = nc.dram_tensor("wiT", [12, 128, 128], BF16, kind="ExternalInput").ap()
    brz_d = nc.dram_tensor("brz", [128, 4], F32, kind="ExternalInput").ap()
    bn_d = nc.dram_tensor("bn", [128, 2], F32, kind="ExternalInput").ap()
    bini_d = nc.dram_tensor("bini", [128, CH * 32], BF16, kind="ExternalInput").ap()
    id_d = nc.dram_tensor("ident", [128, 128], BF16, kind="ExternalInput").ap()
    hout_d = nc.dram_tensor("hout", [128, SCOLS], F32, kind="ExternalOutput").ap()

    with tile.TileContext(nc) as tc:
        consts = tc.alloc_tile_pool(name="consts", bufs=1)
        state = tc.alloc_tile_pool(name="state", bufs=1)
        chunks = tc.alloc_tile_pool(name="chunks", bufs=1)
        psr_pool = tc.alloc_tile_pool(name="psr", bufs=2, space="PSUM")
        psz_pool = tc.alloc_tile_pool(name="psz", bufs=2, space="PSUM")
        psn_pool = tc.alloc_tile_pool(name="psn", bufs=2, space="PSUM")
        psgi_pool = tc.alloc_tile_pool(name="psgi", bufs=2, space="PSUM")

        # ---- static tiles ----
        whT_s = consts.tile([128, 12 * 128], BF16, tag="whT")
        wiT_s = consts.tile([128, 12 * 128], BF16, tag="wiT")
        brz_s = consts.tile([128, 4], F32, tag="brz")
        bn_s = consts.tile([128, 2], F32, tag="bn")
        id_s = consts.tile([128, 128], BF16, tag="ident")
        for t12 in range(12):
            nc.sync.dma_start(whT_s[:, t12 * 128:(t12 + 1) * 128], whT_d[t12])
            nc.sync.dma_start(wiT_s[:, t12 * 128:(t12 + 1) * 128], wiT_d[t12])
        nc.sync.dma_start(brz_s[:], brz_d[:])
        nc.sync.dma_start(bn_s[:], bn_d[:])
        nc.sync.dma_start(id_s[:], id_d[:])

        # ---- persistent ping-pong state tiles ----
        ht = [state.tile([128, 32], BF16, tag=f"ht{i}", name=f"ht{i}")
              for i in range(2)]
        sd0 = [state.tile([128, 32], F32, tag=f"sd0{i}", name=f"sd0{i}")
               for i in range(2)]
        nt = [state.tile([128, 32], F32, tag=f"nt{i}", name=f"nt{i}")
              for i in range(2)]
        zv = [state.tile([128, 32], F32, tag=f"zv{i}", name=f"zv{i}")
              for i in range(2)]
        szt = [state.tile([128, 16], F32, tag=f"szt{i}", name=f"szt{i}")
               for i in range(2)]
        t2b = [state.tile([128, 32], F32, tag=f"t2b{i}", name=f"t2b{i}")
               for i in range(2)]
        hstage = state.tile([128, SCOLS], F32, tag="hstage")
        for i in range(2):
            nc.vector.memset(ht[i][:], 0.0)
            nc.vector.memset(sd0[i][:], 0.0)
            nc.vector.memset(nt[i][:], 0.0)

        # ---- per-parity chunk buffers ----
        girz = [chunks.tile([128, CH * 32], BF16, tag=f"girz{p}", name=f"girz{p}")
                for p in range(2)]
        gn = [chunks.tile([128, CH * 32], BF16, tag=f"gn{p}", name=f"gn{p}")
              for p in range(2)]
        mw_s = [chunks.tile([128, CH * SCOLS], F32, tag=f"mw{p}", name=f"mw{p}")
                for p in range(2)]
        xs = [chunks.tile([128, 2 * CH * BS], BF16, tag=f"xs{p}", name=f"xs{p}")
              for p in range(2)]
        # gn evens = bhhn pattern (constant), odds overwritten per chunk
        for p in range(2):
            nc.sync.dma_start(gn[p][:], bini_d[:])

        def dma_x(par, c):
            off = c * CH * BS
            for half in range(2):
                nc.sync.dma_start(
                    xs[par][:, half * CH * BS:(half + 1) * CH * BS],
                    xt_d[half][:, bass.ds(off, CH * BS)])

        def dma_mw(par, c):
            nc.sync.dma_start(
                mw_s[par][:],
                mw_d[0:1, bass.ds(c * CH * SCOLS, CH * SCOLS)]
                .partition_broadcast(128))

        # gi work items for the chunk living in parity `par`:
        # 96 "mm" items (one [128,128] matmul each, 16 steps of one
        # gate-half) + 48 "cp" items (one DVE tensor_scalar each).
        def gi_items(par):
            mm_q = []
            cp_q = []
            for half_n in range(2):            # 64-step halves of the chunk
                for j in range(6):
                    for q in range(4):         # 16-step quarters
                        pg_box = []

                        def mk_mm(jj, nt, qq, kk, box):
                            def emit():
                                if kk == 0:
                                    pg = psgi_pool.tile([128, 128], F32,
                                                        tag="psgi", name="psgi")
                                    box.append(pg)
                                pg = box[0]
                                off = kk * CH * BS + nt * 512 + qq * 128
                                nc.tensor.matmul(
                                    pg[:],
                                    wiT_s[:, (kk * 6 + jj) * 128:
                                          (kk * 6 + jj + 1) * 128],
                                    xs[par][:, off:off + 128],
                                    start=(kk == 0), stop=(kk == 1),
                                    skip_group_check=True)
                            return emit

                        def mk_cp(jj, nt, qq, box):
                            def emit():
                                pg = box[0]
                                src = pg[:].rearrange("p (s b) -> p s b",
                                                      b=GCOLS)
                                s0 = nt * 64 + qq * 16
                                if jj < 4:     # r0,r1,z0,z1 -> girz
                                    dst = girz[par][:].rearrange(
                                        "p (s g) -> p s g", g=32)[
                                        :, s0:s0 + 16,
                                        jj * GCOLS:(jj + 1) * GCOLS]
                                    scale = 1.0 if jj < 2 else -1.0
                                    nc.vector.tensor_scalar(
                                        dst, src, brz_s[:, jj:jj + 1], scale,
                                        op0=ALU.add, op1=ALU.mult)
                                else:          # n0,n1 -> gn odd cols
                                    jn = jj - 4
                                    dst = gn[par][:].rearrange(
                                        "p (s g x) -> p s g x", g=16, x=2)[
                                        :, s0:s0 + 16,
                                        jn * GCOLS:(jn + 1) * GCOLS, 1]
                                    nc.vector.tensor_scalar(
                                        dst, src, bn_s[:, jn:jn + 1], 1.0,
                                        op0=ALU.add, op1=ALU.mult)
                            return emit

                        for kk in range(2):
                            mm_q.append(mk_mm(j, half_n, q, kk, pg_box))
                        cp_q.append(mk_cp(j, half_n, q, pg_box))
            return mm_q, cp_q

        W = whT_s

        def emit_step(par, cs, s, mm_items, cp_items):
            pr = s % 2
            ps_r = psr_pool.tile([128, SCOLS], F32, tag="psr")
            ps_z = psz_pool.tile([128, SCOLS], F32, tag="psz")
            ps_n = psn_pool.tile([128, 32], F32, tag="psn")
            hv = ht[pr][:].rearrange("p (i x) -> p i x", x=2)
            # identity matmuls: psum init from sbuf (bf16, cheap)
            nc.tensor.matmul(ps_r[:], id_s[:],
                             girz[par][:, cs * 32:cs * 32 + 16],
                             start=True, stop=False, skip_group_check=True)
            nc.tensor.matmul(ps_z[:], id_s[:],
                             girz[par][:, cs * 32 + 16:cs * 32 + 32],
                             start=True, stop=False, skip_group_check=True)
            nc.tensor.matmul(ps_n[:], id_s[:],
                             gn[par][:, cs * 32:(cs + 1) * 32],
                             start=True, stop=False, skip_group_check=True)
            # recurrent matmuls: r first (critical path), then z, then n
            psnv = ps_n[:].rearrange("p (i x) -> p i x", x=2)
            for (j, jj, dst) in ((0, 0, ps_r), (1, 1, ps_r),
                                 (2, 0, ps_z), (3, 1, ps_z)):
                for k in range(2):
                    nc.tensor.matmul(
                        dst[:, jj * GCOLS:(jj + 1) * GCOLS],
                        W[:, (k * 6 + j) * 128:(k * 6 + j + 1) * 128],
                        hv[:, k * GCOLS:(k + 1) * GCOLS, 1],
                        start=False, stop=(k == 1 and j in (1, 3)),
                        skip_group_check=True)
            for j in (4, 5):
                jj = j - 4
                for k in range(2):
                    nc.tensor.matmul(
                        psnv[:, jj * GCOLS:(jj + 1) * GCOLS, 0:1],
                        W[:, (k * 6 + j) * 128:(k * 6 + j + 1) * 128],
                        hv[:, k * GCOLS:(k + 1) * GCOLS, 1:2],
                        start=False, stop=(k == 1 and j == 5),
                        skip_group_check=True)
            for mm in mm_items:
                mm()
            # Act: sig_r -> sd0 odds; sig_z -> szt; (tanh later)
            sd0v = sd0[pr][:].rearrange("p (i x) -> p i x", x=2)
            nc.scalar.activation(sd0v[:, :, 1:2],
                                 ps_r[:].rearrange("p (i x) -> p i x", x=1),
                                 AF.Sigmoid)
            nc.scalar.activation(szt[pr][:], ps_z[:], AF.Sigmoid)
            # DVE: scan1: t2 = sig_r*ghn + gin (odds of t2b)
            nc.vector.tensor_tensor_scan(
                t2b[pr][:], sd0[pr][:], ps_n[:], 0.0,
                op0=ALU.mult, op1=ALU.add)
            # Act: tanh(t2) -> nt odds (sole writer of nt)
            t2v = t2b[pr][:].rearrange("p (i x) -> p i x", x=2)
            ntv = nt[pr][:].rearrange("p (i x) -> p i x", x=2)
            nc.scalar.activation(ntv[:, :, 1:2], t2v[:, :, 1:2], AF.Tanh)
            # DVE: zc = sig_z * (-m*w) -> zv evens
            zvv = zv[pr][:].rearrange("p (i x) -> p i x", x=2)
            mwv = mw_s[par][:, cs * SCOLS:(cs + 1) * SCOLS] \
                .rearrange("p (i x) -> p i x", x=1)
            nc.vector.tensor_tensor(
                zvv[:, :, 0:1],
                szt[pr][:].rearrange("p (i x) -> p i x", x=1),
                mwv, op=ALU.mult)
            # DVE: vv = (zc+1)*h -> zv odds
            nc.vector.scalar_tensor_tensor(
                zvv[:, :, 1:2], zvv[:, :, 0:1], 1.0, hv[:, :, 1:2],
                op0=ALU.add, op1=ALU.mult)
            # DVE: scan2: pairs (0,n)x(zc,vv): h' = n*zc + vv (bf16 odds)
            nc.vector.tensor_tensor_scan(
                ht[1 - pr][:], nt[pr][:], zv[pr][:], 0.0,
                op0=ALU.mult, op1=ALU.add)
            if cp_items:
                with tc.high_priority(offset=-60):
                    for cp in cp_items:
                        cp()

        def emit_chunk(par, s0, gi):
            mm_q, cp_q = gi
            mi = ci = 0
            for cs in range(CH):
                mms = []
                if cs >= 1 and mi < len(mm_q):
                    mms.append(mm_q[mi])
                    mi += 1
                cps = []
                if cs % 2 == 1:
                    while ci < len(cp_q) and ci < mi // 2 and len(cps) < 1:
                        cps.append(cp_q[ci])
                        ci += 1
                emit_step(par, cs, s0 + cs, mms, cps)
            while mi < len(mm_q):
                mm_q[mi]()
                mi += 1
            while ci < len(cp_q):
                cp_q[ci]()
                ci += 1

        # ---- prologue ----
        dma_x(0, 0)
        dma_mw(0, 0)
        if nch > 1:
            dma_x(1, 1)
        mm0, cp0 = gi_items(0)
        for i, mm in enumerate(mm0):
            mm()
            if i % 2 == 1:
                cp0[i // 2]()

        # ---- main loop over chunks ----
        for c in range(nch):
            par = c % 2
            if c + 2 < nch:
                dma_x(par, c + 2)
            if c + 1 < nch:
                dma_mw(1 - par, c + 1)
            gi = gi_items(1 - par) if c + 1 < nch else ([], [])
            emit_chunk(par, c * CH, gi)

        # ---- output: h_true = -h_store ----
        hfv = ht[T % 2][:].rearrange("p (i x) -> p i x", x=2)
        nc.scalar.activation(hstage[:].rearrange("p (i x) -> p i x", x=1),
                             hfv[:, :, 1:2], AF.Copy, scale=-1.0)
        nc.sync.dma_start(hout_d[:], hstage[:])

        for p in (psgi_pool, psn_pool, psz_pool, psr_pool,
                  chunks, state, consts):
            p.release()

    nc.compile()
    return nc


def host_prep(x, att_weights, lengths, W_ih, W_hh, b_ih, b_hh, T):
    """Build per-core input maps."""
    mask = (np.arange(T)[None, :] < np.asarray(lengths)[:, None])
    mwneg = (-(mask * np.asarray(att_weights)[:, :T])).astype(np.float32)

    # h is stored negated: negate r and n weight blocks (z stays positive
    # because ps_z accumulates -a_z).
    Wmod = np.concatenate([-W_hh[0:H], W_hh[H:2 * H], -W_hh[2 * H:3 * H]],
                          axis=0)
    whT = np.zeros((12, 128, 128), np.float32)
    wiT = np.zeros((12, 128, 128), np.float32)
    for k in range(2):
        for j in range(6):
            whT[k * 6 + j] = Wmod[j * 128:(j + 1) * 128, k * 128:(k + 1) * 128].T
            wiT[k * 6 + j] = W_ih[j * 128:(j + 1) * 128, k * 128:(k + 1) * 128].T
    whT = whT.astype(BF16NP)
    wiT = wiT.astype(BF16NP)

    bsum = (b_ih + b_hh).astype(np.float32)
    brz = np.zeros((128, 4), np.float32)
    brz[:, 0] = bsum[0:128]
    brz[:, 1] = bsum[128:256]
    brz[:, 2] = bsum[256:384]
    brz[:, 3] = bsum[384:512]
    bn = np.zeros((128, 2), np.float32)
    bn[:, 0] = b_ih[512:640]
    bn[:, 1] = b_ih[640:768]
    # gn init pattern: evens = bhh_n (per half), odds = 0
    pat = np.zeros((128, 32), np.float32)
    for jj in range(2):
        pat[:, 2 * (jj * GCOLS) + np.arange(GCOLS) * 2] = \
            b_hh[512 + jj * 128:512 + (jj + 1) * 128][:, None]
    bini = np.tile(pat, (1, CH)).astype(BF16NP)
    ident = np.eye(128, dtype=np.float32).astype(BF16NP)

    in_maps = []
    for c in range(NCORES):
        bs = slice(c * BS, (c + 1) * BS)
        xc = np.asarray(x[bs, :T]).transpose(2, 1, 0)       # [I, T, BS]
        xt = xc.reshape(2, 128, T * BS).astype(BF16NP)
        mwc = mwneg[bs].T                                    # [T, BS]
        mwt = np.concatenate([mwc, mwc], axis=1).reshape(1, T * SCOLS)
        in_maps.append({
            "xt": xt, "mw": np.ascontiguousarray(mwt), "whT": whT,
            "wiT": wiT, "brz": brz, "bn": bn, "bini": bini, "ident": ident,
        })
    return in_maps


def assemble_out(results):
    out = np.zeros((B, H), np.float32)
    for c, res in enumerate(results):
        ho = res["hout"]                      # [128, 16]
        for k in range(2):
            out[c * BS:(c + 1) * BS, k * 128:(k + 1) * 128] = \
                ho[:, k * GCOLS:(k + 1) * GCOLS].T
    return out


def run_T(lengths):
    mx = int(np.asarray(lengths).max())
    nch = max(1, -(-mx // CH))
    return min(nch * CH, T_FULL)


def kernel(x, att_weights, lengths, W_ih, W_hh, b_ih, b_hh):
    T = run_T(lengths)
    in_maps = host_prep(np.asarray(x), np.asarray(att_weights),
                        np.asarray(lengths), np.asarray(W_ih),
                        np.asarray(W_hh), np.asarray(b_ih),
                        np.asarray(b_hh), T)
    nc = build_nc(T)
    res = None
    for attempt in range(3):
        try:
            res = run_bass_kernel_spmd(nc, in_maps, core_ids=list(range(NCORES)))
            break
        except Exception:
            if attempt == 2:
                raise
    return assemble_out(res.results)
